# revision 1
# baseline (speedup 1.0000x reference)
"""Trainium2 Bass kernel for AttentionalPositionEncoding.

Reference computation (per batch b, with x_tok = x.reshape(C, N).T):
    cnn   = x_tok @ Wc.T
    q     = cnn @ Wq.T + bq           -> heads [h=8, N=1024, dk=32]
    k     = pos @ Wk.T + bk
    v     = pos @ Wv.T + bv
    attn  = softmax(q k^T / sqrt(dk)) @ v
    out   = (cnn @ Wf.T + bf + attn) @ Wo.T + bo + x_tok

Sharding: data-parallel over B=8 across the 8 NeuronCores (1 batch/core).

Host-side weight folding (exact algebra, done in fp32):
    Wqc  = Wq @ Wc          (q     = x_tok @ Wqc.T + bq)
    Wofc = Wo @ Wf @ Wc     (ffn   = x_tok @ Wofc.T)
    bfo  = Wo @ bf + bo

On-chip layout is feature-major ("CN": features on partitions, tokens on
free dim), which matches the HBM layout of x/pos ([C, H*W]) so no input
transposes are needed.  Attention scores are computed *transposed*
(S^T[j, i], keys on psum partitions) so that exp(S^T) feeds the P@V
matmul directly as the moving operand.  Softmax is unnormalized
(scores are O(8), exp is safe in fp32); the row sums Z are produced by a
ones-column appended to V (M=33 stationary), and 1/Z is applied after
P@V via a small select-matrix broadcast matmul.

The default execution path is the "batched" variant: per head pair, all
32 score matmuls + 16 [128,1024] exps run first (single PE tile mode,
ACT-paced), then all 32 P@V matmuls run back-to-back with contiguous
PSUM accumulation chains.  All matmuls are fp32r (full-rate fp32),
measured end-to-end relative error vs the fp32 reference: 1.6e-4.
Local measurement: ~190 us/kernel on one NeuronCore (8 cores run
data-parallel, one batch each).
"""

import math

import numpy as np

import concourse.bacc as bacc
import concourse.mybir as mybir
import concourse.tile as tile
from concourse.bass_utils import run_bass_kernel_spmd

F32 = mybir.dt.float32
F32R = mybir.dt.float32r
BF16 = mybir.dt.bfloat16

D = 256          # d_model
H = 8            # heads
DK = 32          # head dim
N = 1024         # tokens (32*32)
NCORES = 8
SCALE = 1.0 / math.sqrt(DK)


def _r(ap):
    """Bitcast an AP to float32r so the PE runs at 1 cycle/row."""
    return ap.bitcast(F32R)


def build(loop_input=False, variant="full"):
    """Build the per-core Bass program.

    loop_input=True adds a uint32 [1,1] input "niter" and wraps the whole
    body in a dynamic For_i — used by the local timing harness only.
    variant: "full" | ablations used for local perf attribution.
    """
    nc = bacc.Bacc(None, target_bir_lowering=False)

    x_d = nc.dram_tensor("x", [D, N], F32R, kind="ExternalInput")
    pos_d = nc.dram_tensor("pos", [D, N], F32R, kind="ExternalInput")
    wqcT_d = nc.dram_tensor("wqcT", [D, D], F32R, kind="ExternalInput")
    wkT_d = nc.dram_tensor("wkT", [D, D], F32R, kind="ExternalInput")
    # v weights augmented with a zero 33rd column per head; the ones come
    # from the bias row, so P@V also produces the softmax denominators Z.
    wvT_d = nc.dram_tensor("wvT", [D, H * (DK + 1)], F32R, kind="ExternalInput")
    wofcT_d = nc.dram_tensor("wofcT", [D, D], F32R, kind="ExternalInput")
    woT_d = nc.dram_tensor("woT", [D, D], F32R, kind="ExternalInput")
    bpp_d = nc.dram_tensor("b_pp", [128, 6], F32, kind="ExternalInput")
    brow_d = nc.dram_tensor("b_row", [1, H * (DK + 1)], F32R,
                            kind="ExternalInput")
    ones_d = nc.dram_tensor("ones1", [1, 128], F32R, kind="ExternalInput")
    out_d = nc.dram_tensor("out", [D, N], F32, kind="ExternalOutput")
    if loop_input:
        niter_d = nc.dram_tensor("niter", [1, 1], mybir.dt.uint32,
                                 kind="ExternalInput")

    with tile.TileContext(nc) as tc:
        import contextlib
        with contextlib.ExitStack() as stk:
            if loop_input:
                cpool = stk.enter_context(tc.tile_pool(name="cfg", bufs=1))
                nit_sb = cpool.tile([1, 1], mybir.dt.uint32)
                nc.sync.dma_start(nit_sb[:], niter_d[:])
                nit = nc.values_load(nit_sb[0:1, 0:1], min_val=1,
                                     max_val=1 << 20,
                                     skip_runtime_bounds_check=True)
                loop_cm = tc.For_i(0, nit, 1,
                                   hint_engines=tuple(mybir.ALL_ENGINES))
            else:
                loop_cm = contextlib.nullcontext()
            with loop_cm:
                _body(nc, tc, x_d, pos_d, wqcT_d, wkT_d, wvT_d, wofcT_d,
                      woT_d, bpp_d, brow_d, ones_d, out_d, variant)
    nc.compile()
    return nc


def _body(nc, tc, x_d, pos_d, wqcT_d, wkT_d, wvT_d, wofcT_d, woT_d,
          bpp_d, brow_d, ones_d, out_d, variant="full"):
    import contextlib
    with contextlib.ExitStack() as stk:
        ep = stk.enter_context

        persist = ep(tc.tile_pool(name="persist", bufs=1))

        # ---------- load inputs ----------
        def load_cn(dram, name):
            t = persist.tile([128, 2, dram.shape[1]], F32R, tag=name)
            nc.sync.dma_start(t[:], dram[:].rearrange("(k p) n -> p k n", p=128))
            return t

        x_sb = load_cn(x_d, "x_sb")          # [128, kt, 1024]
        pos_sb = load_cn(pos_d, "pos_sb")
        wqc_sb = load_cn(wqcT_d, "wqc_sb")   # [128, kt, 256]
        wk_sb = load_cn(wkT_d, "wk_sb")
        wv_sb = load_cn(wvT_d, "wv_sb")
        wofc_sb = load_cn(wofcT_d, "wofc_sb")
        wo_sb = load_cn(woT_d, "wo_sb")
        bpp = persist.tile([128, 6], F32, tag="bpp")
        nc.sync.dma_start(bpp[:], bpp_d[:])
        brow = persist.tile([1, H * (DK + 1)], F32R, tag="brow")
        nc.sync.dma_start(brow[:], brow_d[:])

        # constants
        ones1 = persist.tile([1, 128], F32R, tag="ones1")
        nc.sync.dma_start(ones1[:], ones_d[:])
        zbias = persist.tile([128, 1], F32, tag="zbias")
        nc.gpsimd.memset(zbias[:], 0.0)

        # persistent activations
        v2 = variant.startswith("v2") or variant == "pipelined"
        pv_dt = BF16 if ("bf16pv" in variant or v2) else F32R
        qk_dt = BF16 if variant.startswith("v2") else F32R
        q_sb = persist.tile([128, 2, N], qk_dt, tag="q_sb")
        k_sb = persist.tile([128, 2, N], qk_dt, tag="k_sb")
        v_aug = persist.tile([128, 8, H * (DK + 1)], pv_dt, tag="v_aug")
        oT_sb = persist.tile([128, 2, N], F32R, tag="oT_sb")
        # 1/Z rows, all on partition 0 (engine APs need 32-aligned bases)
        zinv = persist.tile([1, H, N], F32R, tag="zinv")
        out_sb = persist.tile([128, 2, N], F32, tag="out_sb")

        # ---------- q / k projections (CN layout) ----------
        with tc.tile_pool(name="dense_ps", bufs=2, space="PSUM") as dense_ps:
            for (dst, w_sb, rhs_sb, bcol) in ((q_sb, wqc_sb, x_sb, 0),
                                              (k_sb, wk_sb, pos_sb, 2)):
                for mt in range(2):
                    for ch in range(2):
                        ps = dense_ps.tile([128, 512], F32, tag="dense")
                        for kt in range(2):
                            nc.tensor.matmul(
                                ps[:],
                                _r(w_sb[:, kt, 128 * mt:128 * mt + 128]),
                                _r(rhs_sb[:, kt, 512 * ch:512 * ch + 512]),
                                start=(kt == 0), stop=(kt == 1))
                        with nc.allow_low_precision(reason="qk dtype knob"):
                            nc.vector.tensor_scalar_add(
                                dst[:, mt, 512 * ch:512 * ch + 512], ps[:],
                                bpp[:, bcol + mt:bcol + mt + 1])

            # ------- v projection (token-major, ones-augmented) -------
            for jt in range(8):
                ps = dense_ps.tile([128, H * (DK + 1)], F32, tag="dense")
                for kt in range(2):
                    nc.tensor.matmul(
                        ps[:],
                        _r(pos_sb[:, kt, 128 * jt:128 * jt + 128]),
                        _r(wv_sb[:, kt, :]),
                        start=(kt == 0), stop=False)
                nc.tensor.matmul(ps[:], _r(ones1[:]), _r(brow[:]),
                                 start=False, stop=True)
                with nc.allow_low_precision(reason="pv dtype knob"):
                    nc.vector.tensor_copy(v_aug[:, jt, :], ps[:])

        # ---------- attention: head pairs ----------
        attn_stk = stk.enter_context(contextlib.ExitStack())
        sc_ps = attn_stk.enter_context(
            tc.tile_pool(name="sc_ps", bufs=2, space="PSUM"))
        pv_ps = attn_stk.enter_context(
            tc.tile_pool(name="pv_ps", bufs=1, space="PSUM"))
        n_et = 34 if variant == "pipelined" else (
            18 if (variant.startswith("batched")
                   or variant.startswith("v2")) else 3)
        e_pool = attn_stk.enter_context(tc.tile_pool(name="e_pool", bufs=n_et))

        if variant.startswith("v2") and variant != "v2noattn":
            # bf16 attention: per-head score tiles with N=1024 streams.
            for hp in range(4):
                hA, hB = 2 * hp, 2 * hp + 1
                dt = hA // 4
                pA, pB = 32 * (hA % 4), 32 * (hB % 4)
                pvA = pv_ps.tile([128, N], F32, tag="pvA")
                pvB = pv_ps.tile([128, N], F32, tag="pvB")
                ets = {}
                # phase 1: scores + exp, one PE mode (32-row tiles)
                for jt in range(8):
                    for (h0, p0) in ((hA, pA), (hB, pB)):
                        sc = sc_ps.tile([128, 1024], F32, tag="sc")
                        nc.tensor.matmul(
                            sc[:],
                            k_sb[p0:p0 + 32, dt, 128 * jt:128 * jt + 128],
                            q_sb[p0:p0 + 32, dt, :],
                            start=True, stop=True, tile_position=(p0, 0))
                        et = e_pool.tile([128, 1024], BF16, tag="et")
                        with nc.allow_low_precision(reason="bf16 attention"):
                            nc.scalar.activation(
                                et[:], sc[:],
                                mybir.ActivationFunctionType.Exp,
                                bias=zbias[:, 0:1], scale=SCALE)
                        ets[(h0, jt)] = et
                # phase 2: P@V, one PE mode, contiguous accum chains
                for (h0, pvt) in ((hA, pvA), (hB, pvB)):
                    for jt in range(8):
                        nc.tensor.matmul(
                            pvt[0:DK + 1, :],
                            v_aug[:, jt,
                                  (DK + 1) * h0:(DK + 1) * h0 + DK + 1],
                            ets[(h0, jt)][:],
                            start=(jt == 0), stop=(jt == 7),
                            tile_position=(0, 0))
                nc.vector.tensor_copy(oT_sb[pA:pA + 32, dt, :], pvA[0:32, :])
                nc.vector.tensor_copy(oT_sb[pB:pB + 32, dt, :], pvB[0:32, :])
                with nc.allow_low_precision(reason="f32r full 32-bit width"):
                    nc.vector.reciprocal(zinv[0:1, hA, :], pvA[32:33, :])
                    nc.vector.reciprocal(zinv[0:1, hB, :], pvB[32:33, :])

        if variant == "pipelined":
            # Software-pipelined pairs: pair p's P@V matmuls are emitted
            # after pair p+1's scores+exp, so the PE fills the ACT-paced
            # stretches with P@V work instead of idling.  E and V in bf16
            # so two pairs of exp outputs fit in SBUF.
            def phase1(hp):
                hA, hB = 2 * hp, 2 * hp + 1
                dt = hA // 4
                pA, pB = 32 * (hA % 4), 32 * (hB % 4)
                ets = []
                for jt in range(8):
                    for ch in range(2):
                        sc = sc_ps.tile([128, 1024], F32, tag="sc")
                        for (h0, p0, lo) in ((hA, pA, 0), (hB, pB, 512)):
                            nc.tensor.matmul(
                                sc[:, lo:lo + 512],
                                _r(k_sb[p0:p0 + 32, dt,
                                        128 * jt:128 * jt + 128]),
                                _r(q_sb[p0:p0 + 32, dt,
                                        512 * ch:512 * ch + 512]),
                                start=True, stop=True, tile_position=(p0, 0))
                        et = e_pool.tile([128, 1024], BF16, tag="et")
                        with nc.allow_low_precision(reason="bf16 E"):
                            nc.scalar.activation(
                                et[:], sc[:],
                                mybir.ActivationFunctionType.Exp,
                                bias=zbias[:, 0:1], scale=SCALE)
                        ets.append(et)
                return ets

            def phase2(hp, ets):
                hA, hB = 2 * hp, 2 * hp + 1
                dt = hA // 4
                pA, pB = 32 * (hA % 4), 32 * (hB % 4)
                pvA = pv_ps.tile([128, N], F32, tag="pvA")
                pvB = pv_ps.tile([128, N], F32, tag="pvB")
                for (h0, elo, pvt) in ((hA, 0, pvA), (hB, 512, pvB)):
                    for ch in range(2):
                        for jt in range(8):
                            nc.tensor.matmul(
                                pvt[0:DK + 1, 512 * ch:512 * ch + 512],
                                v_aug[:, jt,
                                      (DK + 1) * h0:(DK + 1) * h0 + DK + 1],
                                ets[jt * 2 + ch][:, elo:elo + 512],
                                start=(jt == 0), stop=(jt == 7),
                                tile_position=(0, 0))
                nc.vector.tensor_copy(oT_sb[pA:pA + 32, dt, :], pvA[0:32, :])
                nc.vector.tensor_copy(oT_sb[pB:pB + 32, dt, :], pvB[0:32, :])
                with nc.allow_low_precision(reason="f32r full width"):
                    nc.vector.reciprocal(zinv[0:1, hA, :], pvA[32:33, :])
                    nc.vector.reciprocal(zinv[0:1, hB, :], pvB[32:33, :])

            prev = None
            for hp in range(4):
                ets = phase1(hp)
                if prev is not None:
                    phase2(prev[0], prev[1])
                prev = (hp, ets)
            phase2(prev[0], prev[1])

        if variant.startswith("batched"):
            for hp in range(4):
                hA, hB = 2 * hp, 2 * hp + 1
                dt = hA // 4
                pA, pB = 32 * (hA % 4), 32 * (hB % 4)
                pvA = pv_ps.tile([128, N], F32, tag="pvA")
                pvB = pv_ps.tile([128, N], F32, tag="pvB")
                ets = []
                # phase 1: all scores + exp for the pair (one PE mode)
                for jt in range(8):
                    for ch in range(2):
                        sc = sc_ps.tile([128, 1024], F32, tag="sc")
                        for (h0, p0, lo) in ((hA, pA, 0), (hB, pB, 512)):
                            nc.tensor.matmul(
                                sc[:, lo:lo + 512],
                                _r(k_sb[p0:p0 + 32, dt,
                                        128 * jt:128 * jt + 128]),
                                _r(q_sb[p0:p0 + 32, dt,
                                        512 * ch:512 * ch + 512]),
                                start=True, stop=True, tile_position=(p0, 0))
                        et = e_pool.tile([128, 1024], pv_dt, tag="et")
                        with nc.allow_low_precision(reason="pv dtype knob"):
                            nc.scalar.activation(
                                et[:], sc[:],
                                mybir.ActivationFunctionType.Exp,
                                bias=zbias[:, 0:1], scale=SCALE)
                        ets.append(et)
                # phase 2: all P@V for the pair (one PE mode, each psum
                # region's accumulation chain contiguous: BEGIN..MID..END)
                for (h0, elo, pvt) in ((hA, 0, pvA), (hB, 512, pvB)):
                    for ch in range(2):
                        for jt in range(8):
                            et = ets[jt * 2 + ch]
                            grp = ((jt == 0), (jt == 7))
                            if variant == "batched_nogroup":
                                grp = (True, True)
                            nc.tensor.matmul(
                                pvt[0:DK + 1, 512 * ch:512 * ch + 512],
                                v_aug[:, jt,
                                      (DK + 1) * h0:(DK + 1) * h0 + DK + 1],
                                et[:, elo:elo + 512],
                                start=grp[0], stop=grp[1],
                                tile_position=(0, 0))
                nc.vector.tensor_copy(oT_sb[pA:pA + 32, dt, :], pvA[0:32, :])
                nc.vector.tensor_copy(oT_sb[pB:pB + 32, dt, :], pvB[0:32, :])
                with nc.allow_low_precision(reason="f32r full 32-bit width"):
                    nc.vector.reciprocal(zinv[0:1, hA, :], pvA[32:33, :])
                    nc.vector.reciprocal(zinv[0:1, hB, :], pvB[32:33, :])

        for hp in (() if (variant.startswith("batched") or v2) else range(4)):
            hA, hB = 2 * hp, 2 * hp + 1
            dt = hA // 4
            pA, pB = 32 * (hA % 4), 32 * (hB % 4)
            # fp32r matmul dst base partition must be 0 (or 64 with M<=32):
            # give each head its own psum tile, both written at (0, 0), M=33.
            pvA = pv_ps.tile([128, N], F32, tag="pvA")
            pvB = pv_ps.tile([128, N], F32, tag="pvB")
            for jt in range(8):
                for ch in range(2):
                    sc = sc_ps.tile([128, 1024], F32, tag="sc")
                    # S^T tiles for heads A (cols 0:512) and B (cols 512:1024)
                    for (h0, p0, lo) in ((hA, pA, 0), (hB, pB, 512)):
                        nc.tensor.matmul(
                            sc[:, lo:lo + 512],
                            _r(k_sb[p0:p0 + 32, dt, 128 * jt:128 * jt + 128]),
                            _r(q_sb[p0:p0 + 32, dt, 512 * ch:512 * ch + 512]),
                            start=True, stop=True, tile_position=(p0, 0))
                    et = e_pool.tile([128, 1024], F32R, tag="et")
                    if variant == "expdve":
                        with nc.allow_low_precision(reason="perf ablation"):
                            nc.vector.tensor_copy(et[:], sc[:])
                    elif variant == "exp_sbuf":
                        st = e_pool.tile([128, 1024], F32, tag="st")
                        with nc.allow_low_precision(reason="perf ablation"):
                            nc.vector.tensor_copy(st[:], sc[:])
                        nc.scalar.activation(et[:], st[:],
                                             mybir.ActivationFunctionType.Exp,
                                             bias=zbias[:, 0:1], scale=SCALE)
                    else:
                        nc.scalar.activation(et[:], sc[:],
                                             mybir.ActivationFunctionType.Exp,
                                             bias=zbias[:, 0:1], scale=SCALE)
                    # P@V (+Z row at 32): accumulate over jt
                    if variant == "sconly":
                        if jt == 0:
                            for pvt in (pvA, pvB):
                                nc.tensor.matmul(
                                    pvt[0:DK + 1, 512 * ch:512 * ch + 512],
                                    _r(v_aug[:, jt, 0:DK + 1]),
                                    _r(et[:, 0:512]),
                                    start=True, stop=True,
                                    tile_position=(0, 0))
                    else:
                        for (h0, elo, pvt) in ((hA, 0, pvA), (hB, 512, pvB)):
                            grp = ((jt == 0), (jt == 7))
                            if variant == "batched_nogroup":
                                grp = (True, True)
                            nc.tensor.matmul(
                                pvt[0:DK + 1, 512 * ch:512 * ch + 512],
                                v_aug[:, jt,
                                      (DK + 1) * h0:(DK + 1) * h0 + DK + 1],
                                et[:, elo:elo + 512],
                                start=grp[0], stop=grp[1],
                                tile_position=(0, 0))
            # evacuate: unnormalized attn^T (CN) + 1/Z rows
            nc.vector.tensor_copy(oT_sb[pA:pA + 32, dt, :], pvA[0:32, :])
            nc.vector.tensor_copy(oT_sb[pB:pB + 32, dt, :], pvB[0:32, :])
            with nc.allow_low_precision(reason="f32r is full 32-bit width"):
                nc.vector.reciprocal(zinv[0:1, hA, :], pvA[32:33, :])
                nc.vector.reciprocal(zinv[0:1, hB, :], pvB[32:33, :])

        attn_stk.close()

        # ---------- normalize attn^T by 1/Z (K=1 broadcast matmuls) ----------
        # dst base partition must be 0, so broadcast each head-group row into
        # [32, g, 512] psum and multiply with a partition-shifted DVE op.
        z_ps = ep(tc.tile_pool(name="z_ps", bufs=1, space="PSUM"))
        for dt in range(2):
            for ch in range(2):
                zx = z_ps.tile([32, 4, 512], F32, tag="zx")
                for g in range(4):
                    nc.tensor.matmul(
                        zx[0:32, g, :],
                        _r(ones1[0:1, 0:32]),
                        _r(zinv[0:1, 4 * dt + g, 512 * ch:512 * ch + 512]),
                        start=True, stop=True, tile_position=(0, 0))
                for g in range(4):
                    sl = (slice(32 * g, 32 * g + 32), dt,
                          slice(512 * ch, 512 * ch + 512))
                    nc.vector.tensor_mul(oT_sb[sl], oT_sb[sl], zx[0:32, g, :])

        # ---------- output: Wo @ attn^T + Wofc @ x^T + bfo + x ----------
        fin_ps = ep(tc.tile_pool(name="fin_ps", bufs=2, space="PSUM"))
        for ct in range(2):
            for ch in range(2):
                ps = fin_ps.tile([128, 512], F32, tag="fin")
                first = True
                for (w_sb, rhs_sb) in ((wo_sb, oT_sb), (wofc_sb, x_sb)):
                    for kt in range(2):
                        nc.tensor.matmul(
                            ps[:],
                            _r(w_sb[:, kt, 128 * ct:128 * ct + 128]),
                            _r(rhs_sb[:, kt, 512 * ch:512 * ch + 512]),
                            start=first, stop=(w_sb is wofc_sb and kt == 1))
                        first = False
                sl = (slice(None), ct, slice(512 * ch, 512 * ch + 512))
                nc.vector.tensor_add(out_sb[sl], ps[:],
                                     x_sb[:, ct, 512 * ch:512 * ch + 512])
                nc.vector.tensor_scalar_add(out_sb[sl], out_sb[sl],
                                            bpp[:, 4 + ct:4 + ct + 1])
        nc.sync.dma_start(out_d[:].rearrange("(k p) n -> p k n", p=128),
                          out_sb[:])


_CACHE = {}


def _get_nc(loop_input=False, variant="full"):
    key = (loop_input, variant)
    if key not in _CACHE:
        _CACHE[key] = build(loop_input, variant)
    return _CACHE[key]


def make_in_maps(x, pos_code, Wq, bq, Wk, bk, Wv, bv, Wo, bo, Wc, Wf, bf,
                 extra=None):
    x = np.asarray(x, np.float32)
    pos_code = np.asarray(pos_code, np.float32)
    wqcT = np.ascontiguousarray((np.asarray(Wq) @ np.asarray(Wc)).T, np.float32)
    wkT = np.ascontiguousarray(np.asarray(Wk).T, np.float32)
    # augmented V: per head 32 value cols + a zero col (ones come from bias)
    wvT = np.zeros((D, H * (DK + 1)), np.float32)
    brow = np.zeros((1, H * (DK + 1)), np.float32)
    vT = np.asarray(Wv).T
    bv_np = np.asarray(bv, np.float32)
    for h in range(H):
        wvT[:, (DK + 1) * h:(DK + 1) * h + DK] = vT[:, DK * h:DK * h + DK]
        brow[0, (DK + 1) * h:(DK + 1) * h + DK] = bv_np[DK * h:DK * h + DK]
        brow[0, (DK + 1) * h + DK] = 1.0
    wofcT = np.ascontiguousarray(
        (np.asarray(Wo) @ np.asarray(Wf) @ np.asarray(Wc)).T, np.float32)
    woT = np.ascontiguousarray(np.asarray(Wo).T, np.float32)
    bfo = (np.asarray(Wo) @ np.asarray(bf) + np.asarray(bo)).astype(np.float32)
    b_pp = np.stack([np.asarray(bq, np.float32).reshape(2, 128)[0],
                     np.asarray(bq, np.float32).reshape(2, 128)[1],
                     np.asarray(bk, np.float32).reshape(2, 128)[0],
                     np.asarray(bk, np.float32).reshape(2, 128)[1],
                     bfo.reshape(2, 128)[0],
                     bfo.reshape(2, 128)[1]], axis=1)
    b_pp = np.ascontiguousarray(b_pp, np.float32)          # [128, 6]

    B = x.shape[0]
    in_maps = []
    for b in range(B):
        m = {
            "x": np.ascontiguousarray(x[b].reshape(D, N)),
            "pos": np.ascontiguousarray(pos_code[b].reshape(D, N)),
            "wqcT": wqcT, "wkT": wkT, "wvT": wvT, "wofcT": wofcT,
            "woT": woT, "b_pp": b_pp, "b_row": brow,
            "ones1": np.ones((1, 128), np.float32),
        }
        if extra:
            m.update(extra)
        in_maps.append(m)
    return in_maps


def kernel(**inputs):
    nc = _get_nc(False, "batched")
    in_maps = make_in_maps(**inputs)
    res = run_bass_kernel_spmd(nc, in_maps, core_ids=list(range(NCORES)),
                               trace=False)
    out = np.stack([r["out"].reshape(D, N).T for r in res.results], axis=0)
    return np.ascontiguousarray(out, np.float32)



# revision 14
# speedup vs baseline: 1.4108x; 1.4108x over previous
"""Trainium2 Bass kernel for AttentionalPositionEncoding (v3).

Reference computation (per batch b, with x_tok = x.reshape(C, N).T):
    cnn   = x_tok @ Wc.T
    q     = cnn @ Wq.T + bq           -> heads [h=8, N=1024, dk=32]
    k     = pos @ Wk.T + bk
    v     = pos @ Wv.T + bv
    attn  = softmax(q k^T / sqrt(dk)) @ v
    out   = (cnn @ Wf.T + bf + attn) @ Wo.T + bo + x_tok

Sharding: data-parallel over B=8 across the 8 NeuronCores (1 batch/core).

Host-side weight folding (exact algebra, fp32):
    Wqc   = Wq @ Wc                   (q = x_tok @ Wqc.T + bq)
    Wofc' = Wo @ Wf @ Wc + I          (ffn+residual = x_tok @ Wofc'.T)
    bfo   = Wo @ bf + bo
    woTp  = Wo.T rows permuted+zero-padded to the attention pair-tile layout

v3 design (vs the v2 "batched" baseline at ~250us local):
  * q/k/v in bf16. Scores are bf16 matmuls with dk=32 stationaries placed on
    all four PE row groups (tile_position=(32r,0)) so up to 4 run concurrently.
  * The exp of the 8.4M scores is split across BOTH ScalarE (exact Exp ->
    bf16) and VectorE (one-instruction Schraudolph: bf16-bits = int16(
    A16*scale*s + B16), written through an int16 bitcast of the et tile).
    End-to-end rel err of the full approximation stack measured 1.1e-3.
  * P@V is 2-way column-tiled: per head pair, head A accumulates at psum
    partitions 0:33 (tile (0,0)) and head B at 64:97 (tile (0,64)), with the
    ones-augmented 33rd V column producing the softmax denominators Z.
  * Softmax normalization: Z rows are DMA-gathered to adjacent partitions,
    one reciprocal_approx_fast per quad, DMA partition-broadcast back, and a
    single full-width [128,1024] multiply per pair tile (on GPSIMD) -- this
    replaces ~20us of 32-partition-wide DVE work in the baseline.
  * Final projection: Wo is consumed in the permuted pair-tile layout (junk
    partitions hit zero rows), the +x residual is folded into Wofc'.
"""

import math

import numpy as np

import concourse.bacc as bacc
import concourse.mybir as mybir
import concourse.tile as tile
from concourse.bass import AP
from concourse.bass_utils import run_bass_kernel_spmd

F32 = mybir.dt.float32
F32R = mybir.dt.float32r
BF16 = mybir.dt.bfloat16
I16 = mybir.dt.int16

D = 256          # d_model
H = 8            # heads
DK = 32          # head dim
N = 1024         # tokens (32*32)
NCORES = 8
SCALE = 1.0 / math.sqrt(DK)

# Schraudolph constants for bf16-bits exp: i16 = A16*(SCALE*s) + B16
C16 = 44.0
A16 = float(2 ** 7 / math.log(2))
B16 = float(127 * 2 ** 7 - C16)

# Of the 64 exp bursts, this many go to ScalarE (exact exp); the rest go to
# VectorE (Schraudolph).  Balanced against each engine's other work.
N_ACT_BURSTS = 37


def _r(ap):
    return ap.bitcast(F32R)


def _act_bursts():
    """Evenly interleaved choice of which burst indices use ScalarE."""
    take, acc = set(), 0
    for i in range(64):
        acc += N_ACT_BURSTS
        if acc >= 64:
            acc -= 64
            take.add(i)
    return take


def build(loop_input=False, variant="v3"):
    nc = bacc.Bacc(None, target_bir_lowering=False)

    x_d = nc.dram_tensor("x", [D, N], F32R, kind="ExternalInput")
    pos_d = nc.dram_tensor("pos", [D, N], F32R, kind="ExternalInput")
    wqcT_d = nc.dram_tensor("wqcT", [D, D], F32R, kind="ExternalInput")
    wkT_d = nc.dram_tensor("wkT", [D, D], F32R, kind="ExternalInput")
    wvT_d = nc.dram_tensor("wvT", [D, H * (DK + 1)], F32R, kind="ExternalInput")
    wofcT_d = nc.dram_tensor("wofcT", [D, D], F32R, kind="ExternalInput")
    wop_d = nc.dram_tensor("wop", [128, 4 * D], F32R, kind="ExternalInput")
    bpp_d = nc.dram_tensor("b_pp", [128, 6], F32, kind="ExternalInput")
    brow_d = nc.dram_tensor("b_row", [1, H * (DK + 1)], F32R,
                            kind="ExternalInput")
    ones_d = nc.dram_tensor("ones1", [1, 128], F32R, kind="ExternalInput")
    out_d = nc.dram_tensor("out", [D, N], F32, kind="ExternalOutput")
    dbg = {}
    if variant == "v3dbg":
        dbg["q"] = nc.dram_tensor("dbg_q", [128, 2 * N], F32, kind="ExternalOutput")
        dbg["k"] = nc.dram_tensor("dbg_k", [128, 2 * N], F32, kind="ExternalOutput")
        dbg["v"] = nc.dram_tensor("dbg_v", [128, 8 * 264], F32, kind="ExternalOutput")
        dbg["et"] = nc.dram_tensor("dbg_et", [128, 2 * N], F32, kind="ExternalOutput")
        dbg["pvs"] = nc.dram_tensor("dbg_pvs", [128, 4 * N], F32, kind="ExternalOutput")
        dbg["zri"] = nc.dram_tensor("dbg_zri", [64, 2 * N], F32, kind="ExternalOutput")
        dbg["zr"] = nc.dram_tensor("dbg_zr", [64, 2 * N], F32, kind="ExternalOutput")
        dbg["zbc"] = nc.dram_tensor("dbg_zbc", [128, 4 * N], F32, kind="ExternalOutput")
        dbg["oTn"] = nc.dram_tensor("dbg_oTn", [128, 4 * N], F32, kind="ExternalOutput")
    if loop_input:
        niter_d = nc.dram_tensor("niter", [1, 1], mybir.dt.uint32,
                                 kind="ExternalInput")

    with tile.TileContext(nc) as tc:
        import contextlib
        with contextlib.ExitStack() as stk:
            if loop_input:
                cpool = stk.enter_context(tc.tile_pool(name="cfg", bufs=1))
                nit_sb = cpool.tile([1, 1], mybir.dt.uint32)
                nc.sync.dma_start(nit_sb[:], niter_d[:])
                nit = nc.values_load(nit_sb[0:1, 0:1], min_val=1,
                                     max_val=1 << 20,
                                     skip_runtime_bounds_check=True)
                loop_cm = tc.For_i(0, nit, 1,
                                   hint_engines=tuple(mybir.ALL_ENGINES))
            else:
                loop_cm = contextlib.nullcontext()
            with loop_cm:
                _body(nc, tc, x_d, pos_d, wqcT_d, wkT_d, wvT_d, wofcT_d,
                      wop_d, bpp_d, brow_d, ones_d, out_d, variant, dbg)
    nc.compile()
    return nc


def _body(nc, tc, x_d, pos_d, wqcT_d, wkT_d, wvT_d, wofcT_d, wop_d,
          bpp_d, brow_d, ones_d, out_d, variant="v3", dbg=None):
    import contextlib
    with contextlib.ExitStack() as stk:
        ep = stk.enter_context
        Copy = mybir.ActivationFunctionType.Copy
        Ident = mybir.ActivationFunctionType.Identity
        Exp = mybir.ActivationFunctionType.Exp

        persist = ep(tc.tile_pool(name="persist", bufs=1))

        def load_cn(dram, name):
            t = persist.tile([128, 2, dram.shape[1]], F32R, tag=name)
            nc.sync.dma_start(t[:], dram[:].rearrange("(k p) n -> p k n", p=128))
            return t

        x_sb = load_cn(x_d, "x_sb")          # [128, 2, 1024]
        wqc_sb = load_cn(wqcT_d, "wqc_sb")   # [128, 2, 256]
        pos_sb = load_cn(pos_d, "pos_sb")
        wk_sb = load_cn(wkT_d, "wk_sb")
        wv_sb = load_cn(wvT_d, "wv_sb")      # [128, 2, 264]
        wofc_sb = load_cn(wofcT_d, "wofc_sb")
        wop_sb = persist.tile([128, 4, D], F32R, tag="wop_sb")
        nc.sync.dma_start(wop_sb[:], wop_d[:].rearrange("p (t n) -> p t n", t=4))
        bpp = persist.tile([128, 6], F32, tag="bpp")
        nc.sync.dma_start(bpp[:], bpp_d[:])
        brow = persist.tile([1, H * (DK + 1)], F32R, tag="brow")
        nc.sync.dma_start(brow[:], brow_d[:])
        ones1 = persist.tile([1, 128], F32R, tag="ones1")
        nc.sync.dma_start(ones1[:], ones_d[:])
        zbias = persist.tile([128, 1], F32, tag="zbias")
        nc.gpsimd.memset(zbias[:], 0.0)

        # persistent activations
        q_sb = persist.tile([128, 2, N], BF16, tag="q_sb")
        k_sb = persist.tile([128, 2, N], BF16, tag="k_sb")
        v_aug = persist.tile([128, 8, H * (DK + 1)], BF16, tag="v_aug")
        pvs_sb = persist.tile([128, 4, N], F32, tag="pvs_sb")
        zr = persist.tile([64, 2, N], F32, tag="zr")
        zri = persist.tile([64, 2, N], F32, tag="zri")
        zbc = persist.tile([128, 4, N], F32, tag="zbc")
        oTn = persist.tile([128, 4, N], F32R, tag="oTn")
        out_sb = persist.tile([128, 2, N], F32, tag="out_sb")
        et_dbg = None
        if variant == "v3dbg":
            et_dbg = persist.tile([128, 2, N], BF16, tag="et_dbg")

        # ---------- dense projections ----------
        with tc.tile_pool(name="dense_ps", bufs=2, space="PSUM") as dense_ps:
            for (dst, w_sb, rhs_sb, bcol) in ((q_sb, wqc_sb, x_sb, 0),
                                              (k_sb, wk_sb, pos_sb, 2)):
                for mt in range(2):
                    for ch in range(2):
                        ps = dense_ps.tile([128, 512], F32, tag="dense")
                        for kt in range(2):
                            nc.tensor.matmul(
                                ps[:],
                                _r(w_sb[:, kt, 128 * mt:128 * mt + 128]),
                                _r(rhs_sb[:, kt, 512 * ch:512 * ch + 512]),
                                start=(kt == 0), stop=(kt == 1))
                        with nc.allow_low_precision(reason="bf16 qk"):
                            nc.scalar.activation(
                                dst[:, mt, 512 * ch:512 * ch + 512], ps[:],
                                Ident, bias=bpp[:, bcol + mt:bcol + mt + 1],
                                scale=1.0)

            for jt in range(8):
                ps = dense_ps.tile([128, H * (DK + 1)], F32, tag="dense")
                for kt in range(2):
                    nc.tensor.matmul(
                        ps[:],
                        _r(pos_sb[:, kt, 128 * jt:128 * jt + 128]),
                        _r(wv_sb[:, kt, :]),
                        start=(kt == 0), stop=False)
                nc.tensor.matmul(ps[:], _r(ones1[:]), _r(brow[:]),
                                 start=False, stop=True)
                with nc.allow_low_precision(reason="bf16 v"):
                    nc.vector.tensor_copy(v_aug[:, jt, :], ps[:])

        # ---------- attention ----------
        attn_stk = stk.enter_context(contextlib.ExitStack())
        sc_ps = attn_stk.enter_context(
            tc.tile_pool(name="sc_ps", bufs=3, space="PSUM"))
        pv_ps = attn_stk.enter_context(
            tc.tile_pool(name="pv_ps", bufs=1, space="PSUM"))
        e_pool = attn_stk.enter_context(tc.tile_pool(name="e_pool", bufs=6))

        bidx = 0
        for dt in range(2):
            for ch in range(2):
                pvt = []
                for p in range(2):
                    pvtile = pv_ps.tile([128, 512], F32, tag=f"pv{p}")
                    pvt.append(pvtile)
                for jt in range(8):
                    ets = []
                    for pair in range(2):
                        sc = sc_ps.tile([128, 1024], F32, tag="sc")
                        for s in range(2):
                            p0 = 32 * (2 * pair + s)
                            nc.tensor.matmul(
                                sc[:, 512 * s:512 * s + 512],
                                k_sb[p0:p0 + 32, dt,
                                     128 * jt:128 * jt + 128],
                                q_sb[p0:p0 + 32, dt,
                                     512 * ch:512 * ch + 512],
                                start=True, stop=True,
                                tile_position=(p0, 0))
                        et = e_pool.tile([128, 1024], BF16, tag="et")
                        with nc.allow_low_precision(reason="bf16 attn"):
                            if (dt + ch + pair) % 2 == 0:
                                nc.scalar.activation(
                                    et[:], sc[:], Exp,
                                    bias=zbias[:, 0:1], scale=SCALE)
                            else:
                                nc.vector.tensor_scalar(
                                    et[:].bitcast(I16), sc[:],
                                    A16 * SCALE, B16,
                                    mybir.AluOpType.mult,
                                    mybir.AluOpType.add)
                        if variant == "v3dbg" and dt == 0 and ch == 0 \
                                and jt == 0:
                            with nc.allow_low_precision(reason="dbg"):
                                nc.vector.tensor_copy(
                                    et_dbg[:, pair, :], et[:])
                        ets.append(et)
                        bidx += 1
                    for pair in range(2):
                        hA = 4 * dt + 2 * pair
                        et = ets[pair]
                        nc.tensor.matmul(
                            pvt[pair][0:DK + 1, :],
                            v_aug[:, jt, 33 * hA:33 * hA + 33],
                            et[:, 0:512],
                            start=(jt == 0), stop=(jt == 7),
                            tile_position=(0, 0))
                        nc.tensor.matmul(
                            pvt[pair][64:64 + DK + 1, :],
                            v_aug[:, jt, 33 * hA + 33:33 * hA + 66],
                            et[:, 512:1024],
                            start=(jt == 0), stop=(jt == 7),
                            tile_position=(0, 64))
                # evacuate this ch's pv accumulators (alternate engines)
                for pair in range(2):
                    t = 2 * dt + pair
                    dst = pvs_sb[:, t, 512 * ch:512 * ch + 512]
                    if (dt + ch + pair) % 2 == 0:
                        nc.scalar.activation(dst, pvt[pair][:], Copy,
                                             bias=0.0, scale=1.0)
                    else:
                        nc.vector.tensor_copy(dst, pvt[pair][:])
            # quad dt finished: Z gather -> reciprocal -> broadcast -> norm
            for pair in range(2):
                t = 2 * dt + pair
                r = 2 * pair
                nc.sync.dma_start(zr[r:r + 1, dt, :], pvs_sb[32:33, t, :])
                nc.sync.dma_start(zr[r + 1:r + 2, dt, :], pvs_sb[96:97, t, :])
            nc.vector.reciprocal_approx_fast(zri[0:4, dt, :],
                                             zr[0:4, dt, :])
            for pair in range(2):
                t = 2 * dt + pair
                r = 2 * pair
                for s in range(2):
                    zsrc = zri[r + s:r + s + 1, dt, :]
                    zsrc = AP(zsrc.tensor, zsrc.offset,
                              [list(zsrc.ap[0]), [0, 64], [1, N]])
                    nc.sync.dma_start(zbc[64 * s:64 * s + 64, t, :], zsrc)
                with nc.allow_low_precision(reason="f32r round for PE"):
                    nc.gpsimd.tensor_mul(oTn[:, t, :], pvs_sb[:, t, :],
                                         zbc[:, t, :])

        attn_stk.close()

        # ---------- final projection ----------
        fin_ps = ep(tc.tile_pool(name="fin_ps", bufs=2, space="PSUM"))
        for ct in range(2):
            for ch in range(2):
                ps = fin_ps.tile([128, 512], F32, tag="fin")
                for t in range(4):
                    nc.tensor.matmul(
                        ps[:],
                        wop_sb[:, t, 128 * ct:128 * ct + 128],
                        oTn[:, t, 512 * ch:512 * ch + 512],
                        start=(t == 0), stop=False)
                for kt in range(2):
                    nc.tensor.matmul(
                        ps[:],
                        _r(wofc_sb[:, kt, 128 * ct:128 * ct + 128]),
                        _r(x_sb[:, kt, 512 * ch:512 * ch + 512]),
                        start=False, stop=(kt == 1))
                sl = (slice(None), ct, slice(512 * ch, 512 * ch + 512))
                nc.scalar.activation(out_sb[sl], ps[:], Ident,
                                     bias=bpp[:, 4 + ct:4 + ct + 1], scale=1.0)
        nc.sync.dma_start(out_d[:].rearrange("(k p) n -> p k n", p=128),
                          out_sb[:])
        if variant == "v3dbg":
            dq = persist.tile([128, 2, N], F32, tag="dq")
            dk_ = persist.tile([128, 2, N], F32, tag="dk_")
            dv = persist.tile([128, 8, 264], F32, tag="dv")
            det = persist.tile([128, 2, N], F32, tag="det")
            for (dstt, srct) in ((dq, q_sb), (dk_, k_sb), (dv, v_aug),
                                 (det, et_dbg)):
                nc.vector.tensor_copy(dstt[:], srct[:])
            nc.sync.dma_start(dbg["q"][:].rearrange("p (k n) -> p k n", k=2), dq[:])
            nc.sync.dma_start(dbg["k"][:].rearrange("p (k n) -> p k n", k=2), dk_[:])
            nc.sync.dma_start(dbg["v"][:].rearrange("p (k n) -> p k n", k=8), dv[:])
            nc.sync.dma_start(dbg["et"][:].rearrange("p (k n) -> p k n", k=2), det[:])
            nc.sync.dma_start(dbg["pvs"][:].rearrange("p (k n) -> p k n", k=4), pvs_sb[:])
            nc.sync.dma_start(dbg["zri"][:].rearrange("p (k n) -> p k n", k=2), zri[:])
            nc.sync.dma_start(dbg["zr"][:].rearrange("p (k n) -> p k n", k=2), zr[:])
            nc.sync.dma_start(dbg["zbc"][:].rearrange("p (k n) -> p k n", k=4), zbc[:])
            nc.sync.dma_start(dbg["oTn"][:].rearrange("p (k n) -> p k n", k=4), oTn[:].bitcast(F32))


_CACHE = {}


def _get_nc(loop_input=False, variant="v3"):
    key = (loop_input, variant)
    if key not in _CACHE:
        _CACHE[key] = build(loop_input, variant)
    return _CACHE[key]


def make_in_maps(x, pos_code, Wq, bq, Wk, bk, Wv, bv, Wo, bo, Wc, Wf, bf,
                 extra=None):
    x = np.asarray(x, np.float32)
    pos_code = np.asarray(pos_code, np.float32)
    wqcT = np.ascontiguousarray((np.asarray(Wq) @ np.asarray(Wc)).T, np.float32)
    wkT = np.ascontiguousarray(np.asarray(Wk).T, np.float32)
    wvT = np.zeros((D, H * (DK + 1)), np.float32)
    brow = np.zeros((1, H * (DK + 1)), np.float32)
    vT = np.asarray(Wv).T
    bv_np = np.asarray(bv, np.float32)
    for h in range(H):
        wvT[:, 33 * h:33 * h + DK] = vT[:, DK * h:DK * h + DK]
        brow[0, 33 * h:33 * h + DK] = bv_np[DK * h:DK * h + DK]
        brow[0, 33 * h + DK] = 1.0
    wofcT = np.ascontiguousarray(
        (np.asarray(Wo) @ np.asarray(Wf) @ np.asarray(Wc)
         + np.eye(D, dtype=np.float64)).T, np.float32)
    # permuted Wo for the pair-tile layout: tile t=2*dt+pair holds head
    # hA=2t rows at partitions 0:32 and head hB=2t+1 rows at 64:96.
    woT = np.asarray(Wo).T.astype(np.float32)          # [attn_dim, 256]
    wop = np.zeros((128, 4, D), np.float32)
    for t in range(4):
        wop[0:32, t, :] = woT[32 * (2 * t):32 * (2 * t) + 32, :]
        wop[64:96, t, :] = woT[32 * (2 * t + 1):32 * (2 * t + 1) + 32, :]
    wop = np.ascontiguousarray(wop.reshape(128, 4 * D))
    bfo = (np.asarray(Wo) @ np.asarray(bf) + np.asarray(bo)).astype(np.float32)
    b_pp = np.stack([np.asarray(bq, np.float32).reshape(2, 128)[0],
                     np.asarray(bq, np.float32).reshape(2, 128)[1],
                     np.asarray(bk, np.float32).reshape(2, 128)[0],
                     np.asarray(bk, np.float32).reshape(2, 128)[1],
                     bfo.reshape(2, 128)[0],
                     bfo.reshape(2, 128)[1]], axis=1)
    b_pp = np.ascontiguousarray(b_pp, np.float32)          # [128, 6]

    B = x.shape[0]
    in_maps = []
    for b in range(B):
        m = {
            "x": np.ascontiguousarray(x[b].reshape(D, N)),
            "pos": np.ascontiguousarray(pos_code[b].reshape(D, N)),
            "wqcT": wqcT, "wkT": wkT, "wvT": wvT, "wofcT": wofcT,
            "wop": wop, "b_pp": b_pp, "b_row": brow,
            "ones1": np.ones((1, 128), np.float32),
        }
        if extra:
            m.update(extra)
        in_maps.append(m)
    return in_maps


def kernel(**inputs):
    nc = _get_nc(False, "v3")
    in_maps = make_in_maps(**inputs)
    res = run_bass_kernel_spmd(nc, in_maps, core_ids=list(range(NCORES)),
                               trace=False)
    out = np.stack([r["out"].reshape(D, N).T for r in res.results], axis=0)
    return np.ascontiguousarray(out, np.float32)


# revision 16
# speedup vs baseline: 2.4724x; 1.7525x over previous
"""Trainium2 Bass kernel for AttentionalPositionEncoding (v3).

Reference computation (per batch b, with x_tok = x.reshape(C, N).T):
    cnn   = x_tok @ Wc.T
    q     = cnn @ Wq.T + bq           -> heads [h=8, N=1024, dk=32]
    k     = pos @ Wk.T + bk
    v     = pos @ Wv.T + bv
    attn  = softmax(q k^T / sqrt(dk)) @ v
    out   = (cnn @ Wf.T + bf + attn) @ Wo.T + bo + x_tok

Sharding: data-parallel over B=8 across the 8 NeuronCores (1 batch/core).

Host-side weight folding (exact algebra, fp32):
    Wqc   = Wq @ Wc                   (q = x_tok @ Wqc.T + bq)
    Wofc' = Wo @ Wf @ Wc + I          (ffn+residual = x_tok @ Wofc'.T)
    bfo   = Wo @ bf + bo
    woTp  = Wo.T rows permuted+zero-padded to the attention pair-tile layout

v3 design (vs the v2 "batched" baseline at ~250us local):
  * q/k/v in bf16. Scores are bf16 matmuls with dk=32 stationaries placed on
    all four PE row groups (tile_position=(32r,0)) so up to 4 run concurrently.
  * The exp of the 8.4M scores is split across BOTH ScalarE (exact Exp ->
    bf16) and VectorE (one-instruction Schraudolph: bf16-bits = int16(
    A16*scale*s + B16), written through an int16 bitcast of the et tile).
    End-to-end rel err of the full approximation stack measured 1.1e-3.
  * P@V is 2-way column-tiled: per head pair, head A accumulates at psum
    partitions 0:33 (tile (0,0)) and head B at 64:97 (tile (0,64)), with the
    ones-augmented 33rd V column producing the softmax denominators Z.
  * Softmax normalization: Z rows are DMA-gathered to adjacent partitions,
    one reciprocal_approx_fast per quad, DMA partition-broadcast back, and a
    single full-width [128,1024] multiply per pair tile (on GPSIMD) -- this
    replaces ~20us of 32-partition-wide DVE work in the baseline.
  * Final projection: Wo is consumed in the permuted pair-tile layout (junk
    partitions hit zero rows), the +x residual is folded into Wofc'.
"""

import math

import numpy as np

import concourse.bacc as bacc
import concourse.mybir as mybir
import concourse.tile as tile
from concourse.bass import AP
from concourse.bass_utils import run_bass_kernel_spmd

F32 = mybir.dt.float32
F32R = mybir.dt.float32r
BF16 = mybir.dt.bfloat16
I16 = mybir.dt.int16

D = 256          # d_model
H = 8            # heads
DK = 32          # head dim
N = 1024         # tokens (32*32)
NCORES = 8
SCALE = 1.0 / math.sqrt(DK)

# Schraudolph constants for bf16-bits exp: i16 = A16*(SCALE*s) + B16
C16 = 44.0
A16 = float(2 ** 7 / math.log(2))
B16 = float(127 * 2 ** 7 - C16)

# Of the 64 exp bursts, this many go to ScalarE (exact exp); the rest go to
# VectorE (Schraudolph).  Balanced against each engine's other work.
N_ACT_BURSTS = 37


def _r(ap):
    return ap.bitcast(F32R)


def _act_bursts():
    """Evenly interleaved choice of which burst indices use ScalarE."""
    take, acc = set(), 0
    for i in range(64):
        acc += N_ACT_BURSTS
        if acc >= 64:
            acc -= 64
            take.add(i)
    return take


def build(loop_input=False, variant="v3"):
    nc = bacc.Bacc(None, target_bir_lowering=False)

    x_d = nc.dram_tensor("x", [D, N], F32R, kind="ExternalInput")
    pos_d = nc.dram_tensor("pos", [D, N], F32R, kind="ExternalInput")
    wqcT_d = nc.dram_tensor("wqcT", [D, D], F32R, kind="ExternalInput")
    wkT_d = nc.dram_tensor("wkT", [D, D], F32R, kind="ExternalInput")
    wvT_d = nc.dram_tensor("wvT", [D, H * (DK + 1)], F32R, kind="ExternalInput")
    wofcT_d = nc.dram_tensor("wofcT", [D, D], F32R, kind="ExternalInput")
    wop_d = nc.dram_tensor("wop", [128, 4 * D], F32R, kind="ExternalInput")
    bpp_d = nc.dram_tensor("b_pp", [128, 6], F32, kind="ExternalInput")
    brow_d = nc.dram_tensor("b_row", [1, H * (DK + 1)], F32R,
                            kind="ExternalInput")
    ones_d = nc.dram_tensor("ones1", [1, 128], F32R, kind="ExternalInput")
    out_d = nc.dram_tensor("out", [D, N], F32, kind="ExternalOutput")
    dbg = {}
    if variant == "v3dbg":
        dbg["q"] = nc.dram_tensor("dbg_q", [128, 2 * N], F32, kind="ExternalOutput")
        dbg["k"] = nc.dram_tensor("dbg_k", [128, 2 * N], F32, kind="ExternalOutput")
        dbg["v"] = nc.dram_tensor("dbg_v", [128, 8 * 264], F32, kind="ExternalOutput")
        dbg["et"] = nc.dram_tensor("dbg_et", [128, 2 * N], F32, kind="ExternalOutput")
        dbg["pvs"] = nc.dram_tensor("dbg_pvs", [128, 4 * N], F32, kind="ExternalOutput")
        dbg["zri"] = nc.dram_tensor("dbg_zri", [64, 2 * N], F32, kind="ExternalOutput")
        dbg["zr"] = nc.dram_tensor("dbg_zr", [64, 2 * N], F32, kind="ExternalOutput")
        dbg["zbc"] = nc.dram_tensor("dbg_zbc", [128, 4 * N], F32, kind="ExternalOutput")
        dbg["oTn"] = nc.dram_tensor("dbg_oTn", [128, 4 * N], F32, kind="ExternalOutput")
    if loop_input:
        niter_d = nc.dram_tensor("niter", [1, 1], mybir.dt.uint32,
                                 kind="ExternalInput")

    with tile.TileContext(nc) as tc:
        import contextlib
        with contextlib.ExitStack() as stk:
            if loop_input:
                cpool = stk.enter_context(tc.tile_pool(name="cfg", bufs=1))
                nit_sb = cpool.tile([1, 1], mybir.dt.uint32)
                nc.sync.dma_start(nit_sb[:], niter_d[:])
                nit = nc.values_load(nit_sb[0:1, 0:1], min_val=1,
                                     max_val=1 << 20,
                                     skip_runtime_bounds_check=True)
                loop_cm = tc.For_i(0, nit, 1,
                                   hint_engines=tuple(mybir.ALL_ENGINES))
            else:
                loop_cm = contextlib.nullcontext()
            with loop_cm:
                _body(nc, tc, x_d, pos_d, wqcT_d, wkT_d, wvT_d, wofcT_d,
                      wop_d, bpp_d, brow_d, ones_d, out_d, variant, dbg)
    nc.compile()
    return nc


def _body(nc, tc, x_d, pos_d, wqcT_d, wkT_d, wvT_d, wofcT_d, wop_d,
          bpp_d, brow_d, ones_d, out_d, variant="v3", dbg=None):
    import contextlib
    with contextlib.ExitStack() as stk:
        ep = stk.enter_context
        Copy = mybir.ActivationFunctionType.Copy
        Ident = mybir.ActivationFunctionType.Identity
        Exp = mybir.ActivationFunctionType.Exp

        persist = ep(tc.tile_pool(name="persist", bufs=1))

        def load_cn(dram, name):
            t = persist.tile([128, 2, dram.shape[1]], F32R, tag=name)
            nc.sync.dma_start(t[:], dram[:].rearrange("(k p) n -> p k n", p=128))
            return t

        x_sb = load_cn(x_d, "x_sb")          # [128, 2, 1024]
        wqc_sb = load_cn(wqcT_d, "wqc_sb")   # [128, 2, 256]
        pos_sb = load_cn(pos_d, "pos_sb")
        wk_sb = load_cn(wkT_d, "wk_sb")
        wv_sb = load_cn(wvT_d, "wv_sb")      # [128, 2, 264]
        wofc_sb = load_cn(wofcT_d, "wofc_sb")
        wop_sb = persist.tile([128, 4, D], F32R, tag="wop_sb")
        nc.sync.dma_start(wop_sb[:], wop_d[:].rearrange("p (t n) -> p t n", t=4))
        bpp = persist.tile([128, 6], F32, tag="bpp")
        nc.sync.dma_start(bpp[:], bpp_d[:])
        brow = persist.tile([1, H * (DK + 1)], F32R, tag="brow")
        nc.sync.dma_start(brow[:], brow_d[:])
        ones1 = persist.tile([1, 128], F32R, tag="ones1")
        nc.sync.dma_start(ones1[:], ones_d[:])
        zbias = persist.tile([128, 1], F32, tag="zbias")
        nc.gpsimd.memset(zbias[:], 0.0)

        sconly = variant == "v3_sconly"
        noz = variant in ("v3_noz", "v3_sconly")
        # persistent activations
        q_sb = persist.tile([128, 2, N], BF16, tag="q_sb")
        k_sb = persist.tile([128, 2, N], BF16, tag="k_sb")
        v_aug = persist.tile([128, 8, H * (DK + 1)], BF16, tag="v_aug")
        pvs_sb = persist.tile([128, 4, N], F32, tag="pvs_sb")
        zr = persist.tile([64, 2, N], F32, tag="zr")
        zri = persist.tile([64, 2, N], F32, tag="zri")
        zbc = persist.tile([128, 4, N], F32, tag="zbc")
        oTn = persist.tile([128, 4, N], F32R, tag="oTn")
        out_sb = persist.tile([128, 2, N], F32, tag="out_sb")
        if noz:
            with nc.allow_low_precision(reason="ablation"):
                nc.gpsimd.memset(oTn[:].bitcast(F32), 0.0)
        et_dbg = None
        if variant == "v3dbg":
            et_dbg = persist.tile([128, 2, N], BF16, tag="et_dbg")

        # ---------- dense projections ----------
        with tc.tile_pool(name="dense_ps", bufs=2, space="PSUM") as dense_ps:
            for (dst, w_sb, rhs_sb, bcol) in ((q_sb, wqc_sb, x_sb, 0),
                                              (k_sb, wk_sb, pos_sb, 2)):
                for mt in range(2):
                    for ch in range(2):
                        ps = dense_ps.tile([128, 512], F32, tag="dense")
                        for kt in range(2):
                            nc.tensor.matmul(
                                ps[:],
                                _r(w_sb[:, kt, 128 * mt:128 * mt + 128]),
                                _r(rhs_sb[:, kt, 512 * ch:512 * ch + 512]),
                                start=(kt == 0), stop=(kt == 1))
                        with nc.allow_low_precision(reason="bf16 qk"):
                            nc.scalar.activation(
                                dst[:, mt, 512 * ch:512 * ch + 512], ps[:],
                                Ident, bias=bpp[:, bcol + mt:bcol + mt + 1],
                                scale=1.0)

            for jt in range(8):
                ps = dense_ps.tile([128, H * (DK + 1)], F32, tag="dense")
                for kt in range(2):
                    nc.tensor.matmul(
                        ps[:],
                        _r(pos_sb[:, kt, 128 * jt:128 * jt + 128]),
                        _r(wv_sb[:, kt, :]),
                        start=(kt == 0), stop=False)
                nc.tensor.matmul(ps[:], _r(ones1[:]), _r(brow[:]),
                                 start=False, stop=True)
                with nc.allow_low_precision(reason="bf16 v"):
                    nc.vector.tensor_copy(v_aug[:, jt, :], ps[:])

        # ---------- attention ----------
        attn_stk = stk.enter_context(contextlib.ExitStack())
        sc_ps = attn_stk.enter_context(
            tc.tile_pool(name="sc_ps", bufs=3, space="PSUM"))
        pv_ps = attn_stk.enter_context(
            tc.tile_pool(name="pv_ps", bufs=1, space="PSUM"))
        e_pool = attn_stk.enter_context(tc.tile_pool(name="e_pool", bufs=6))

        bidx = 0
        for dt in range(2):
            for ch in range(2):
                pvt = []
                for p in range(2):
                    pvtile = pv_ps.tile([128, 512], F32, tag=f"pv{p}")
                    pvt.append(pvtile)
                for jt in range(8):
                    ets = []
                    for pair in range(2):
                        sc = sc_ps.tile([128, 1024], F32, tag="sc")
                        for s in range(2):
                            p0 = 32 * (2 * pair + s)
                            nc.tensor.matmul(
                                sc[:, 512 * s:512 * s + 512],
                                k_sb[p0:p0 + 32, dt,
                                     128 * jt:128 * jt + 128],
                                q_sb[p0:p0 + 32, dt,
                                     512 * ch:512 * ch + 512],
                                start=True, stop=True,
                                tile_position=(p0, 0))
                        et = e_pool.tile([128, 1024], BF16, tag="et")
                        with nc.allow_low_precision(reason="bf16 attn"):
                            if (dt + ch + pair) % 2 == 0:
                                nc.scalar.activation(
                                    et[:], sc[:], Exp,
                                    bias=zbias[:, 0:1], scale=SCALE)
                            else:
                                nc.vector.tensor_scalar(
                                    et[:].bitcast(I16), sc[:],
                                    A16 * SCALE, B16,
                                    mybir.AluOpType.mult,
                                    mybir.AluOpType.add)
                        if variant == "v3dbg" and dt == 0 and ch == 0 \
                                and jt == 0:
                            with nc.allow_low_precision(reason="dbg"):
                                nc.vector.tensor_copy(
                                    et_dbg[:, pair, :], et[:])
                        ets.append(et)
                        bidx += 1
                    for pair in range(2):
                        if sconly:
                            break
                        hA = 4 * dt + 2 * pair
                        et = ets[pair]
                        nc.tensor.matmul(
                            pvt[pair][0:DK + 1, :],
                            v_aug[:, jt, 33 * hA:33 * hA + 33],
                            et[:, 0:512],
                            start=(jt == 0), stop=(jt == 7),
                            tile_position=(0, 0))
                        nc.tensor.matmul(
                            pvt[pair][64:64 + DK + 1, :],
                            v_aug[:, jt, 33 * hA + 33:33 * hA + 66],
                            et[:, 512:1024],
                            start=(jt == 0), stop=(jt == 7),
                            tile_position=(0, 64))
                # evacuate this ch's pv accumulators (alternate engines)
                for pair in range(2 if not sconly else 0):
                    t = 2 * dt + pair
                    dst = pvs_sb[:, t, 512 * ch:512 * ch + 512]
                    if (dt + ch + pair) % 2 == 0:
                        nc.scalar.activation(dst, pvt[pair][:], Copy,
                                             bias=0.0, scale=1.0)
                    else:
                        nc.vector.tensor_copy(dst, pvt[pair][:])
            # quad dt finished: Z gather -> reciprocal -> broadcast -> norm
            for pair in range(2 if not noz else 0):
                t = 2 * dt + pair
                r = 2 * pair
                nc.sync.dma_start(zr[r:r + 1, dt, :], pvs_sb[32:33, t, :])
                nc.sync.dma_start(zr[r + 1:r + 2, dt, :], pvs_sb[96:97, t, :])
            if not noz:
                nc.vector.reciprocal_approx_fast(zri[0:4, dt, :],
                                                 zr[0:4, dt, :])
            for pair in range(2 if not noz else 0):
                t = 2 * dt + pair
                r = 2 * pair
                for s in range(2):
                    zsrc = zri[r + s:r + s + 1, dt, :]
                    zsrc = AP(zsrc.tensor, zsrc.offset,
                              [list(zsrc.ap[0]), [0, 64], [1, N]])
                    nc.sync.dma_start(zbc[64 * s:64 * s + 64, t, :], zsrc)
                with nc.allow_low_precision(reason="f32r round for PE"):
                    nc.gpsimd.tensor_mul(oTn[:, t, :], pvs_sb[:, t, :],
                                         zbc[:, t, :])

        attn_stk.close()

        # ---------- final projection ----------
        fin_ps = ep(tc.tile_pool(name="fin_ps", bufs=2, space="PSUM"))
        for ct in range(2):
            for ch in range(2):
                ps = fin_ps.tile([128, 512], F32, tag="fin")
                for t in range(4):
                    nc.tensor.matmul(
                        ps[:],
                        wop_sb[:, t, 128 * ct:128 * ct + 128],
                        oTn[:, t, 512 * ch:512 * ch + 512],
                        start=(t == 0), stop=False)
                for kt in range(2):
                    nc.tensor.matmul(
                        ps[:],
                        _r(wofc_sb[:, kt, 128 * ct:128 * ct + 128]),
                        _r(x_sb[:, kt, 512 * ch:512 * ch + 512]),
                        start=False, stop=(kt == 1))
                sl = (slice(None), ct, slice(512 * ch, 512 * ch + 512))
                nc.scalar.activation(out_sb[sl], ps[:], Ident,
                                     bias=bpp[:, 4 + ct:4 + ct + 1], scale=1.0)
        nc.sync.dma_start(out_d[:].rearrange("(k p) n -> p k n", p=128),
                          out_sb[:])
        if variant == "v3dbg":
            dq = persist.tile([128, 2, N], F32, tag="dq")
            dk_ = persist.tile([128, 2, N], F32, tag="dk_")
            dv = persist.tile([128, 8, 264], F32, tag="dv")
            det = persist.tile([128, 2, N], F32, tag="det")
            for (dstt, srct) in ((dq, q_sb), (dk_, k_sb), (dv, v_aug),
                                 (det, et_dbg)):
                nc.vector.tensor_copy(dstt[:], srct[:])
            nc.sync.dma_start(dbg["q"][:].rearrange("p (k n) -> p k n", k=2), dq[:])
            nc.sync.dma_start(dbg["k"][:].rearrange("p (k n) -> p k n", k=2), dk_[:])
            nc.sync.dma_start(dbg["v"][:].rearrange("p (k n) -> p k n", k=8), dv[:])
            nc.sync.dma_start(dbg["et"][:].rearrange("p (k n) -> p k n", k=2), det[:])
            nc.sync.dma_start(dbg["pvs"][:].rearrange("p (k n) -> p k n", k=4), pvs_sb[:])
            nc.sync.dma_start(dbg["zri"][:].rearrange("p (k n) -> p k n", k=2), zri[:])
            nc.sync.dma_start(dbg["zr"][:].rearrange("p (k n) -> p k n", k=2), zr[:])
            nc.sync.dma_start(dbg["zbc"][:].rearrange("p (k n) -> p k n", k=4), zbc[:])
            nc.sync.dma_start(dbg["oTn"][:].rearrange("p (k n) -> p k n", k=4), oTn[:].bitcast(F32))


_CACHE = {}


def _get_nc(loop_input=False, variant="v3"):
    key = (loop_input, variant)
    if key not in _CACHE:
        _CACHE[key] = build(loop_input, variant)
    return _CACHE[key]


def make_in_maps(x, pos_code, Wq, bq, Wk, bk, Wv, bv, Wo, bo, Wc, Wf, bf,
                 extra=None):
    x = np.asarray(x, np.float32)
    pos_code = np.asarray(pos_code, np.float32)
    wqcT = np.ascontiguousarray((np.asarray(Wq) @ np.asarray(Wc)).T, np.float32)
    wkT = np.ascontiguousarray(np.asarray(Wk).T, np.float32)
    wvT = np.zeros((D, H * (DK + 1)), np.float32)
    brow = np.zeros((1, H * (DK + 1)), np.float32)
    vT = np.asarray(Wv).T
    bv_np = np.asarray(bv, np.float32)
    for h in range(H):
        wvT[:, 33 * h:33 * h + DK] = vT[:, DK * h:DK * h + DK]
        brow[0, 33 * h:33 * h + DK] = bv_np[DK * h:DK * h + DK]
        brow[0, 33 * h + DK] = 1.0
    wofcT = np.ascontiguousarray(
        (np.asarray(Wo) @ np.asarray(Wf) @ np.asarray(Wc)
         + np.eye(D, dtype=np.float64)).T, np.float32)
    # permuted Wo for the pair-tile layout: tile t=2*dt+pair holds head
    # hA=2t rows at partitions 0:32 and head hB=2t+1 rows at 64:96.
    woT = np.asarray(Wo).T.astype(np.float32)          # [attn_dim, 256]
    wop = np.zeros((128, 4, D), np.float32)
    for t in range(4):
        wop[0:32, t, :] = woT[32 * (2 * t):32 * (2 * t) + 32, :]
        wop[64:96, t, :] = woT[32 * (2 * t + 1):32 * (2 * t + 1) + 32, :]
    wop = np.ascontiguousarray(wop.reshape(128, 4 * D))
    bfo = (np.asarray(Wo) @ np.asarray(bf) + np.asarray(bo)).astype(np.float32)
    b_pp = np.stack([np.asarray(bq, np.float32).reshape(2, 128)[0],
                     np.asarray(bq, np.float32).reshape(2, 128)[1],
                     np.asarray(bk, np.float32).reshape(2, 128)[0],
                     np.asarray(bk, np.float32).reshape(2, 128)[1],
                     bfo.reshape(2, 128)[0],
                     bfo.reshape(2, 128)[1]], axis=1)
    b_pp = np.ascontiguousarray(b_pp, np.float32)          # [128, 6]

    B = x.shape[0]
    in_maps = []
    for b in range(B):
        m = {
            "x": np.ascontiguousarray(x[b].reshape(D, N)),
            "pos": np.ascontiguousarray(pos_code[b].reshape(D, N)),
            "wqcT": wqcT, "wkT": wkT, "wvT": wvT, "wofcT": wofcT,
            "wop": wop, "b_pp": b_pp, "b_row": brow,
            "ones1": np.ones((1, 128), np.float32),
        }
        if extra:
            m.update(extra)
        in_maps.append(m)
    return in_maps


def kernel(**inputs):
    nc = _get_nc(False, "v3")
    in_maps = make_in_maps(**inputs)
    res = run_bass_kernel_spmd(nc, in_maps, core_ids=list(range(NCORES)),
                               trace=False)
    out = np.stack([r["out"].reshape(D, N).T for r in res.results], axis=0)
    return np.ascontiguousarray(out, np.float32)
